# revision 8
# baseline (speedup 1.0000x reference)
"""2-layer GCN (PyG GCNConv + BN + ReLU) on 8 Trainium2 NeuronCores — v2.

Strategy (node sharding; all exchange in bf16):
  - Nodes sorted by in-degree (desc), dealt round-robin to 8 cores; each
    core owns S slots (real + dead all-zero rows). Self-loops are added as
    ordinary edges (one per slot), so the aggregation psum IS the full
    pre-activation: no separate self-loop term.
  - Per layer: each core computes hs = (a @ W)*dinv[src] for its shard in
    bf16, AllGather -> full bf16 table [G,128] in DRAM (ncfw, ~250us).
  - Edges partitioned by destination core, grouped into 128-dst windows;
    per 128-edge tile: dma_gather bf16 rows (256B each), build
    P[slot,dst] = (dstloc==iota) * dinv[dst] in ONE bf16 tensor_scalar
    (4x DVE mode), accumulate gathered^T @ P into PSUM [128 feat, 128 dst].
  - Window drain on ACT: Copy psum->pre (bf16) with accum_out=sum, then
    Square with accum_out=sumsq. BN stats via tiny AllReduce (~14us);
    y = relu(pre*s + t) in ONE whole-row ACT op (per-partition scale/bias).
  - Output returned transposed [128, S] f32 per core; host reassembles.

dma_gather indices are int16 (<32768): the G-row table is addressed via
two overlapping views lo=[0,32768) / hi=[G-32768,G); segment padding
points at dead rows (core0/core7 tail slots, always padding => hs rows 0).
b1/b2 dropped (BN right after +b is invariant to per-feature shifts).
"""

import numpy as np

C = 8            # cores
D = 128          # feature dim
WIN = 128        # dst nodes per aggregation window (psum free dim)
CH = 8           # max gather tiles (of 128 slots) per dma_gather call
IDX_LIMIT = 32768
REPEAT = 1       # dev knob: repeat the 2-layer body for slope timing
ABLATE_GATHER = False   # dev: skip dma_gather (wrong results, timing only)
ABLATE_P = False        # dev: skip P tensor_scalar build
ABLATE_MM = False       # dev: skip aggregation matmuls
ABLATE_AG = False       # dev: skip the table AllGather
ABLATE_AR = False       # dev: skip the BN AllReduce (wrong stats, timing only)

_cache = {}


def _plan(edge_index, N):
    """Host-side graph preprocessing -> per-core arrays + static structure."""
    src = np.asarray(edge_index[0], dtype=np.int64)
    dst = np.asarray(edge_index[1], dtype=np.int64)
    E = src.shape[0]

    Nr = -(-N // C)                      # real nodes per core
    S = -(-Nr // WIN) * WIN              # padded slots per core
    NW = S // WIN
    HN = S // 128
    G = C * S                            # table rows
    assert G > IDX_LIMIT and S - Nr >= 1, (G, S, Nr)

    deg = np.bincount(dst, minlength=N).astype(np.int64) + 1
    order = np.argsort(-deg, kind="stable")        # rank -> old id
    ranks = np.arange(N, dtype=np.int64)
    g_of_old = np.empty(N, dtype=np.int64)
    g_of_old[order] = (ranks % C) * S + ranks // C

    dinv = np.zeros(G, dtype=np.float64)
    dinv[g_of_old] = deg.astype(np.float64) ** -0.5

    # self-loops as ordinary edges: one per slot (padding slots too; their
    # rows are dead/zero and dinv=0 so they contribute exactly 0)
    gall = np.arange(G, dtype=np.int64)
    gs = np.concatenate([g_of_old[src], gall])
    gd = np.concatenate([g_of_old[dst], gall])

    core_e = gd // S
    w_e = (gd % S) // WIN
    dl_e = (gd % S) % WIN
    view_e = (gs >= IDX_LIMIT).astype(np.int64)
    idx_e = np.where(view_e == 0, gs, gs - (G - IDX_LIMIT))
    assert idx_e.max() < IDX_LIMIT and idx_e.min() >= 0
    val_e = dinv[gd]                     # dinv[dst] folded into the one-hot

    pad_idx = (S - 1, IDX_LIMIT - 1)     # core0 dead row (lo) / core7 dead (hi)

    counts = np.zeros((C, NW, 2), dtype=np.int64)
    np.add.at(counts, (core_e, w_e, view_e), 1)
    K = -(-counts.max(axis=0) // 128)             # [NW, 2] unified tile counts

    # zigzag the view order per window so same-view runs merge across
    # window boundaries (fewer dma_gather calls: chunks break on view change)
    tile_w, tile_v = [], []
    for w in range(NW):
        vorder = (0, 1) if w % 2 == 0 else (1, 0)
        for v in vorder:
            tile_w += [w] * int(K[w, v]); tile_v += [v] * int(K[w, v])
    tile_w = np.array(tile_w, dtype=np.int64)
    tile_v = np.array(tile_v, dtype=np.int64)
    TOTK = len(tile_w)
    TOTS = TOTK * 128

    chunks = []   # (tile0, ntiles, view) — runs of one view, <= CH tiles
    t = 0
    while t < TOTK:
        v = tile_v[t]
        r = t
        while r < TOTK and tile_v[r] == v and r - t < CH:
            r += 1
        chunks.append((int(t), int(r - t), int(v)))
        t = r
    first_tile, last_tile = {}, {}
    for t in range(TOTK):
        w = int(tile_w[t])
        first_tile.setdefault(w, t)
        last_tile[w] = t
    assert len(first_tile) == NW          # every window has >=1 tile

    tile_base = np.zeros((NW, 2), dtype=np.int64)
    b = 0
    for w in range(NW):
        vorder = (0, 1) if w % 2 == 0 else (1, 0)
        for v in vorder:
            tile_base[w, v] = b
            b += int(K[w, v]) * 128
    slot_view = np.empty(TOTS, dtype=np.int64)
    for w in range(NW):
        for v in range(2):
            s0 = int(tile_base[w, v]); n = int(K[w, v]) * 128
            slot_view[s0:s0 + n] = v

    ord_e = np.lexsort((dl_e, view_e, w_e, core_e))
    src_s = idx_e[ord_e]; core_s = core_e[ord_e]
    w_s = w_e[ord_e]; v_s = view_e[ord_e]; dl_s = dl_e[ord_e]
    val_s = val_e[ord_e]

    idx_all = np.empty((C, TOTS), dtype=np.int16)
    dst_all = np.zeros((C, TOTS), dtype=np.float32)
    val_all = np.zeros((C, TOTS), dtype=np.float32)
    for c in range(C):
        m = core_s == c
        iw, iv, ii, idl, ival = w_s[m], v_s[m], src_s[m], dl_s[m], val_s[m]
        arr_i = np.full(TOTS, -1, dtype=np.int64)
        arr_d = np.zeros(TOTS, dtype=np.int64)
        arr_v = np.zeros(TOTS, dtype=np.float64)
        if len(iw):
            seg_id = iw * 2 + iv
            change = np.r_[True, np.diff(seg_id) != 0]
            seg_start = np.maximum.accumulate(
                np.where(change, np.arange(len(seg_id)), 0))
            within = np.arange(len(seg_id)) - seg_start
            flat = tile_base[iw, iv] + within
            arr_i[flat] = ii
            arr_d[flat] = idl
            arr_v[flat] = ival
        padm = arr_i == -1
        arr_i[padm] = np.where(slot_view[padm] == 0, pad_idx[0], pad_idx[1])
        idx_all[c] = arr_i.astype(np.int16)
        dst_all[c] = arr_d.astype(np.float32)
        val_all[c] = arr_v.astype(np.float32)

    idx_wrap = np.empty((C, 128, TOTS // 16), dtype=np.int16)
    for c in range(C):
        idx_wrap[c] = np.tile(idx_all[c].reshape(-1, 16).T, (8, 1))
    # [128, TOTK]: [p, t] = value for slot t*128+p
    dstloc = dst_all.reshape(C, TOTK, 128).transpose(0, 2, 1).copy()
    dvdst = val_all.reshape(C, TOTK, 128).transpose(0, 2, 1).copy()

    dinv_f = dinv.astype(np.float32).reshape(C, S)
    dinv_cols = dinv_f.reshape(C, HN, 128).transpose(0, 2, 1).copy()

    return dict(
        N=N, E=E, S=S, NW=NW, HN=HN, G=G, TOTK=TOTK, TOTS=TOTS,
        K=K, chunks=chunks, tile_w=tile_w, tile_v=tile_v,
        first_tile=first_tile, last_tile=last_tile,
        g_of_old=g_of_old, idx_wrap=idx_wrap, dstloc=dstloc, dvdst=dvdst,
        dinv_cols=dinv_cols,
    )


def _build(tc, outs, ins, plan):
    """Emit the Tile program. ins/outs: dicts of DRAM APs."""
    import contextlib

    import concourse.mybir as mybir

    nc = tc.nc
    S, NW, G, TOTK = plan["S"], plan["NW"], plan["G"], plan["TOTK"]
    N, HN = plan["N"], plan["HN"]
    f32 = mybir.dt.float32
    bf16 = mybir.dt.bfloat16
    i16 = mybir.dt.int16
    AF = mybir.ActivationFunctionType
    OP = mybir.AluOpType
    rg = [list(range(C))]

    hs_dram = [nc.dram_tensor(f"hs{l}", [S, D], bf16) for l in range(2)]
    tables_r = [[nc.dram_tensor(f"table{l}_{r}", [G, D], bf16,
                                addr_space="Shared")
                 for l in range(2)] for r in range(REPEAT)]
    bnin = [nc.dram_tensor(f"bnin{l}", [128, 2], f32) for l in range(2)]
    bnout = [nc.dram_tensor(f"bnout{l}", [128, 2], f32, addr_space="Shared")
             for l in range(2)]

    ctx = contextlib.ExitStack()
    with ctx:
        persist = ctx.enter_context(tc.tile_pool(name="persist", bufs=1))
        gpool = ctx.enter_context(tc.tile_pool(name="gather", bufs=3))
        ppool = ctx.enter_context(tc.tile_pool(name="ptiles", bufs=4))
        spool = ctx.enter_context(tc.tile_pool(name="scratch", bufs=3))
        rowp = ctx.enter_context(tc.tile_pool(name="rows", bufs=3))
        psum_agg = ctx.enter_context(tc.tile_pool(name="psagg", bufs=4, space="PSUM"))
        psum_mm = ctx.enter_context(tc.tile_pool(name="psmm", bufs=2, space="PSUM"))

        aT = persist.tile([128, S], bf16, tag="aT")      # x -> a1
        pre = persist.tile([128, S], bf16, tag="pre")
        osb = persist.tile([128, S], f32, tag="osb")     # final f32 out
        dloc = persist.tile([128, TOTK], f32, tag="dloc")
        dval = persist.tile([128, TOTK], f32, tag="dval")
        idxs = persist.tile([128, plan["TOTS"] // 16], i16, tag="idxs")
        iota = persist.tile([128, WIN], bf16, tag="iota")
        Wt = [persist.tile([128, D], bf16, tag=f"W{l}", name=f"Wt{l}")
              for l in range(2)]
        dcols = persist.tile([128, HN], f32, tag="dcols")
        gbs = persist.tile([128, 4], f32, tag="gbs")
        stats = persist.tile([128, 2 * NW], f32, tag="stats")
        bnsb = [persist.tile([128, 2], f32, tag=f"bnsb{l}", name=f"bnsb{l}")
                for l in range(2)]
        svec = [persist.tile([128, 1], f32, tag=f"svec{l}", name=f"svec{l}")
                for l in range(2)]
        tvec = [persist.tile([128, 1], f32, tag=f"tvec{l}", name=f"tvec{l}")
                for l in range(2)]

        nc.sync.dma_start(out=aT[:], in_=ins["xT"][:, :])
        nc.sync.dma_start(out=dloc[:], in_=ins["dstloc"][:, :])
        nc.sync.dma_start(out=dval[:], in_=ins["dvdst"][:, :])
        nc.sync.dma_start(out=idxs[:], in_=ins["idxw"][:, :])
        nc.sync.dma_start(out=iota[:], in_=ins["iota"][:, :])
        nc.sync.dma_start(out=dcols[:], in_=ins["dinv_cols"][:, :])
        nc.sync.dma_start(out=gbs[:], in_=ins["gb"][:, :])
        nc.sync.dma_start(out=Wt[0][:], in_=ins["W1"][:, :])
        nc.sync.dma_start(out=Wt[1][:], in_=ins["W2"][:, :])

        def phase_mm(l, src_tile):
            """hs{l} = (a @ W) * dinv[src] rows -> DRAM, bf16."""
            for h in range(HN):
                sl = slice(h * 128, (h + 1) * 128)
                psA = psum_mm.tile([128, D], f32, tag="psA")
                nc.tensor.matmul(out=psA[:], lhsT=src_tile[:, sl], rhs=Wt[l][:],
                                 start=True, stop=True)
                hrow = rowp.tile([128, D], bf16, tag="hrow")
                nc.vector.tensor_scalar(out=hrow[:], in0=psA[:],
                                        scalar1=dcols[:, h:h + 1], scalar2=None,
                                        op0=OP.mult)
                nc.sync.dma_start(out=hs_dram[l][sl, :], in_=hrow[:])

        def win_drain(w, psw_w):
            wsl = slice(w * WIN, (w + 1) * WIN)
            nc.scalar.activation(out=pre[:, wsl], in_=psw_w[:], func=AF.Copy,
                                 accum_out=stats[:, w:w + 1])
            sq = spool.tile([128, WIN], bf16, tag="sq")
            nc.scalar.activation(out=sq[:], in_=pre[:, wsl], func=AF.Square,
                                 accum_out=stats[:, NW + w:NW + w + 1])

        def phase_agg(l):
            lo_ap = table[l][0:IDX_LIMIT, :]
            hi_ap = table[l][G - IDX_LIMIT:G, :]
            psw = {}
            for ci, (t0, nt, v) in enumerate(plan["chunks"]):
                n_idx = nt * 128
                gb = gpool.tile([128, CH * 128], bf16, tag="gbuf")
                if not ABLATE_GATHER:
                    out3d = gb[:, :n_idx].rearrange("p (k f) -> p k f", f=D)
                    nc.gpsimd.dma_gather(
                        out_ap=out3d,
                        in_ap=(lo_ap if v == 0 else hi_ap),
                        idxs_ap=idxs[:, t0 * 8:(t0 + nt) * 8],
                        num_idxs=n_idx, num_idxs_reg=n_idx, elem_size=D,
                    )
                else:
                    nc.vector.memset(gb[:, 0:1], 0.0)
                for k in range(nt):
                    t = t0 + k
                    w = int(plan["tile_w"][t])
                    if plan["first_tile"][w] == t:
                        psw[w] = psum_agg.tile([128, WIN], f32, tag="psw",
                                               name=f"psw{w}")
                    P = ppool.tile([128, WIN], bf16, tag="P")
                    if not ABLATE_P:
                        nc.vector.tensor_scalar(
                            out=P[:], in0=iota[:], scalar1=dloc[:, t:t + 1],
                            scalar2=dval[:, t:t + 1], op0=OP.is_equal, op1=OP.mult)
                    else:
                        nc.vector.memset(P[:, 0:1], 0.0)
                    if not ABLATE_MM:
                        nc.tensor.matmul(
                            out=psw[w][:],
                            lhsT=gb[:, k * 128:(k + 1) * 128],
                            rhs=P[:],
                            start=(plan["first_tile"][w] == t),
                            stop=(plan["last_tile"][w] == t))
                    elif plan["first_tile"][w] == t:
                        nc.vector.memset(psw[w][:], 0.0)
                    if plan["last_tile"][w] == t:
                        win_drain(w, psw.pop(w))

        def phase_bn(l, dst_tile):
            ssum = spool.tile([128, 1], f32, tag="bns")
            ssq = spool.tile([128, 1], f32, tag="bnq")
            nc.vector.tensor_reduce(out=ssum[:], in_=stats[:, 0:NW],
                                    axis=mybir.AxisListType.X, op=OP.add)
            nc.vector.tensor_reduce(out=ssq[:], in_=stats[:, NW:2 * NW],
                                    axis=mybir.AxisListType.X, op=OP.add)
            pk = spool.tile([128, 2], f32, tag="bnpack")
            nc.vector.tensor_copy(out=pk[:, 0:1], in_=ssum[:])
            nc.vector.tensor_copy(out=pk[:, 1:2], in_=ssq[:])
            nc.sync.dma_start(out=bnin[l][:, :], in_=pk[:])
            if not ABLATE_AR:
                nc.gpsimd.collective_compute(
                    "AllReduce", OP.add, replica_groups=rg,
                    ins=[bnin[l].ap()], outs=[bnout[l].ap()])
                nc.sync.dma_start(out=bnsb[l][:], in_=bnout[l][:, :])
            else:
                nc.sync.dma_start(out=bnsb[l][:], in_=bnin[l][:, :])
            st = bnsb[l]
            mean = spool.tile([128, 1], f32, tag="mean")
            var = spool.tile([128, 1], f32, tag="var")
            nc.vector.tensor_scalar(out=mean[:], in0=st[:, 0:1],
                                    scalar1=1.0 / N, scalar2=None, op0=OP.mult)
            nc.vector.tensor_scalar(out=var[:], in0=st[:, 1:2],
                                    scalar1=1.0 / N, scalar2=None, op0=OP.mult)
            m2 = spool.tile([128, 1], f32, tag="m2")
            nc.vector.tensor_tensor(out=m2[:], in0=mean[:], in1=mean[:],
                                    op=OP.mult)
            nc.vector.tensor_sub(out=var[:], in0=var[:], in1=m2[:])
            nc.vector.tensor_scalar(out=var[:], in0=var[:], scalar1=1e-5,
                                    scalar2=None, op0=OP.add)
            sd = spool.tile([128, 1], f32, tag="sd")
            nc.scalar.sqrt(out=sd[:], in_=var[:])
            rsd = spool.tile([128, 1], f32, tag="rsd")
            nc.vector.reciprocal(out=rsd[:], in_=sd[:])
            nc.vector.tensor_tensor(out=svec[l][:], in0=gbs[:, 2 * l:2 * l + 1],
                                    in1=rsd[:], op=OP.mult)
            ms = spool.tile([128, 1], f32, tag="ms")
            nc.vector.tensor_tensor(out=ms[:], in0=mean[:], in1=svec[l][:],
                                    op=OP.mult)
            nc.vector.tensor_sub(out=tvec[l][:], in0=gbs[:, 2 * l + 1:2 * l + 2],
                                 in1=ms[:])
            nc.scalar.activation(out=dst_tile[:, :], in_=pre[:, :],
                                 func=AF.Relu, bias=tvec[l][:], scale=svec[l][:])

        for _r in range(REPEAT):
            table = tables_r[_r]
            # layer 1
            phase_mm(0, aT)
            if not ABLATE_AG:
                nc.gpsimd.collective_compute(
                    "AllGather", OP.bypass, replica_groups=rg,
                    ins=[hs_dram[0].ap()], outs=[table[0].ap()])
            phase_agg(0)
            phase_bn(0, aT)                # aT := a1 (bf16)
            # layer 2
            phase_mm(1, aT)
            if not ABLATE_AG:
                nc.gpsimd.collective_compute(
                    "AllGather", OP.bypass, replica_groups=rg,
                    ins=[hs_dram[1].ap()], outs=[table[1].ap()])
            phase_agg(1)
            phase_bn(1, osb)               # osb := relu(bn(pre2)) f32
        nc.sync.dma_start(out=outs["outT"][:, :], in_=osb[:])


def _np_bf16():
    import ml_dtypes
    return np.dtype(ml_dtypes.bfloat16)


def _make_inputs(plan, x, W1, W2, gamma1, beta1, gamma2, beta2):
    S, G = plan["S"], plan["G"]
    bf = _np_bf16()
    xg = np.zeros((G, D), dtype=np.float32)
    xg[plan["g_of_old"]] = np.asarray(x, dtype=np.float32)
    iota = np.tile(np.arange(WIN, dtype=np.float32), (128, 1)).astype(bf)
    gb = np.stack([np.asarray(gamma1, np.float32), np.asarray(beta1, np.float32),
                   np.asarray(gamma2, np.float32), np.asarray(beta2, np.float32)],
                  axis=1).copy()
    in_maps = []
    for c in range(C):
        in_maps.append({
            "xT": np.ascontiguousarray(xg[c * S:(c + 1) * S].T).astype(bf),
            "idxw": plan["idx_wrap"][c],
            "dstloc": plan["dstloc"][c],
            "dvdst": plan["dvdst"][c],
            "dinv_cols": plan["dinv_cols"][c],
            "W1": np.ascontiguousarray(np.asarray(W1, np.float32)).astype(bf),
            "W2": np.ascontiguousarray(np.asarray(W2, np.float32)).astype(bf),
            "gb": gb, "iota": iota,
        })
    return in_maps


def _declare_io(nc, plan):
    import concourse.mybir as mybir
    f32, i16 = mybir.dt.float32, mybir.dt.int16
    bf16 = mybir.dt.bfloat16
    S, TOTK, TOTS, HN = plan["S"], plan["TOTK"], plan["TOTS"], plan["HN"]
    ins = {
        "xT": nc.dram_tensor("xT", [128, S], bf16, kind="ExternalInput"),
        "idxw": nc.dram_tensor("idxw", [128, TOTS // 16], i16,
                               kind="ExternalInput"),
        "dstloc": nc.dram_tensor("dstloc", [128, TOTK], f32,
                                 kind="ExternalInput"),
        "dvdst": nc.dram_tensor("dvdst", [128, TOTK], f32,
                                kind="ExternalInput"),
        "dinv_cols": nc.dram_tensor("dinv_cols", [128, HN], f32,
                                    kind="ExternalInput"),
        "W1": nc.dram_tensor("W1", [128, D], bf16, kind="ExternalInput"),
        "W2": nc.dram_tensor("W2", [128, D], bf16, kind="ExternalInput"),
        "gb": nc.dram_tensor("gb", [128, 4], f32, kind="ExternalInput"),
        "iota": nc.dram_tensor("iota", [128, WIN], bf16, kind="ExternalInput"),
    }
    outs = {"outT": nc.dram_tensor("outT", [128, S], f32,
                                   kind="ExternalOutput")}
    return ins, outs


def _compile(plan, trace_sim=False):
    from concourse import bacc, tile

    nc = bacc.Bacc("TRN2", target_bir_lowering=False, debug=False,
                   num_devices=C)
    ins, outs = _declare_io(nc, plan)
    with tile.TileContext(nc, trace_sim=trace_sim) as tc:
        _build(tc, outs, ins, plan)
    nc.compile()
    return nc


def _run(x, edge_index, W1, b1, gamma1, beta1, W2, b2, gamma2, beta2,
         trace=False):
    from concourse.bass_utils import run_bass_kernel_spmd

    N = x.shape[0]
    plan = _plan(np.asarray(edge_index), N)
    key = (N, plan["E"], plan["TOTK"], tuple(int(k) for k in plan["K"].ravel()))
    if key not in _cache:
        _cache.clear()
        _cache[key] = _compile(plan)
    nc = _cache[key]
    in_maps = _make_inputs(plan, x, W1, W2, gamma1, beta1, gamma2, beta2)
    res = run_bass_kernel_spmd(nc, in_maps, core_ids=list(range(C)), trace=trace)
    S, G = plan["S"], plan["G"]
    aT_full = np.concatenate([res.results[c]["outT"] for c in range(C)], axis=1)
    assert aT_full.shape == (128, G)
    y = np.ascontiguousarray(aT_full.T[plan["g_of_old"]], dtype=np.float32)
    return y, res


def kernel(**inputs):
    y, _ = _run(**inputs)
    return y


# revision 9
# speedup vs baseline: 28.9566x; 28.9566x over previous
"""2-layer GCN (PyG GCNConv + BN + ReLU) on 8 Trainium2 NeuronCores — v2.

Strategy (node sharding; all exchange in bf16):
  - Nodes sorted by in-degree (desc), dealt round-robin to 8 cores; each
    core owns S slots (real + dead all-zero rows). Self-loops are added as
    ordinary edges (one per slot), so the aggregation psum IS the full
    pre-activation: no separate self-loop term.
  - Per layer: each core computes hs = (a @ W)*dinv[src] for its shard in
    bf16, AllGather -> full bf16 table [G,128] in DRAM (ncfw, ~250us).
  - Edges partitioned by destination core, grouped into 128-dst windows;
    per 128-edge tile: dma_gather bf16 rows (256B each), build
    P[slot,dst] = (dstloc==iota) * dinv[dst] in ONE bf16 tensor_scalar
    (4x DVE mode), accumulate gathered^T @ P into PSUM [128 feat, 128 dst].
  - Window drain on ACT: Copy psum->pre (bf16) with accum_out=sum, then
    Square with accum_out=sumsq. BN stats via tiny AllReduce (~14us);
    y = relu(pre*s + t) in ONE whole-row ACT op (per-partition scale/bias).
  - Output returned transposed [128, S] f32 per core; host reassembles.

dma_gather indices are int16 (<32768): the G-row table is addressed via
two overlapping views lo=[0,32768) / hi=[G-32768,G); segment padding
points at dead rows (core0/core7 tail slots, always padding => hs rows 0).
b1/b2 dropped (BN right after +b is invariant to per-feature shifts).
"""

import numpy as np

C = 8            # cores
D = 128          # feature dim
WIN = 128        # dst nodes per aggregation window (psum free dim)
CH = 8           # max gather tiles (of 128 slots) per dma_gather call
IDX_LIMIT = 32768
REPEAT = 1       # dev knob: repeat the 2-layer body for slope timing
ABLATE_GATHER = False   # dev: skip dma_gather (wrong results, timing only)
ABLATE_P = False        # dev: skip P tensor_scalar build
ABLATE_MM = False       # dev: skip aggregation matmuls
ABLATE_AG = False       # dev: skip the table AllGather
ABLATE_AR = False       # dev: skip the BN AllReduce (wrong stats, timing only)

_cache = {}


def _plan(edge_index, N):
    """Host-side graph preprocessing -> per-core arrays + static structure."""
    src = np.asarray(edge_index[0], dtype=np.int64)
    dst = np.asarray(edge_index[1], dtype=np.int64)
    E = src.shape[0]

    Nr = -(-N // C)                      # real nodes per core
    S = -(-Nr // WIN) * WIN              # padded slots per core
    NW = S // WIN
    HN = S // 128
    G = C * S                            # table rows
    assert G > IDX_LIMIT and S - Nr >= 1, (G, S, Nr)

    deg = np.bincount(dst, minlength=N).astype(np.int64) + 1
    order = np.argsort(-deg, kind="stable")        # rank -> old id
    ranks = np.arange(N, dtype=np.int64)
    g_of_old = np.empty(N, dtype=np.int64)
    g_of_old[order] = (ranks % C) * S + ranks // C

    dinv = np.zeros(G, dtype=np.float64)
    dinv[g_of_old] = deg.astype(np.float64) ** -0.5

    # self-loops as ordinary edges: one per slot (padding slots too; their
    # rows are dead/zero and dinv=0 so they contribute exactly 0)
    gall = np.arange(G, dtype=np.int64)
    gs = np.concatenate([g_of_old[src], gall])
    gd = np.concatenate([g_of_old[dst], gall])

    core_e = gd // S
    w_e = (gd % S) // WIN
    dl_e = (gd % S) % WIN
    view_e = (gs >= IDX_LIMIT).astype(np.int64)
    idx_e = np.where(view_e == 0, gs, gs - (G - IDX_LIMIT))
    assert idx_e.max() < IDX_LIMIT and idx_e.min() >= 0
    val_e = dinv[gd]                     # dinv[dst] folded into the one-hot

    pad_idx = (S - 1, IDX_LIMIT - 1)     # core0 dead row (lo) / core7 dead (hi)

    counts = np.zeros((C, NW, 2), dtype=np.int64)
    np.add.at(counts, (core_e, w_e, view_e), 1)
    K = -(-counts.max(axis=0) // 128)             # [NW, 2] unified tile counts

    tile_w, tile_v = [], []
    for w in range(NW):
        tile_w += [w] * int(K[w, 0]); tile_v += [0] * int(K[w, 0])
        tile_w += [w] * int(K[w, 1]); tile_v += [1] * int(K[w, 1])
    tile_w = np.array(tile_w, dtype=np.int64)
    tile_v = np.array(tile_v, dtype=np.int64)
    TOTK = len(tile_w)
    TOTS = TOTK * 128

    chunks = []   # (tile0, ntiles, view) — runs of one view, <= CH tiles
    t = 0
    while t < TOTK:
        v = tile_v[t]
        r = t
        while r < TOTK and tile_v[r] == v and r - t < CH:
            r += 1
        chunks.append((int(t), int(r - t), int(v)))
        t = r
    first_tile, last_tile = {}, {}
    for t in range(TOTK):
        w = int(tile_w[t])
        first_tile.setdefault(w, t)
        last_tile[w] = t
    assert len(first_tile) == NW          # every window has >=1 tile

    tile_base = np.zeros((NW, 2), dtype=np.int64)
    b = 0
    for w in range(NW):
        for v in range(2):
            tile_base[w, v] = b
            b += int(K[w, v]) * 128
    slot_view = np.empty(TOTS, dtype=np.int64)
    for w in range(NW):
        for v in range(2):
            s0 = int(tile_base[w, v]); n = int(K[w, v]) * 128
            slot_view[s0:s0 + n] = v

    ord_e = np.lexsort((dl_e, view_e, w_e, core_e))
    src_s = idx_e[ord_e]; core_s = core_e[ord_e]
    w_s = w_e[ord_e]; v_s = view_e[ord_e]; dl_s = dl_e[ord_e]
    val_s = val_e[ord_e]

    idx_all = np.empty((C, TOTS), dtype=np.int16)
    dst_all = np.zeros((C, TOTS), dtype=np.float32)
    val_all = np.zeros((C, TOTS), dtype=np.float32)
    for c in range(C):
        m = core_s == c
        iw, iv, ii, idl, ival = w_s[m], v_s[m], src_s[m], dl_s[m], val_s[m]
        arr_i = np.full(TOTS, -1, dtype=np.int64)
        arr_d = np.zeros(TOTS, dtype=np.int64)
        arr_v = np.zeros(TOTS, dtype=np.float64)
        if len(iw):
            seg_id = iw * 2 + iv
            change = np.r_[True, np.diff(seg_id) != 0]
            seg_start = np.maximum.accumulate(
                np.where(change, np.arange(len(seg_id)), 0))
            within = np.arange(len(seg_id)) - seg_start
            flat = tile_base[iw, iv] + within
            arr_i[flat] = ii
            arr_d[flat] = idl
            arr_v[flat] = ival
        padm = arr_i == -1
        arr_i[padm] = np.where(slot_view[padm] == 0, pad_idx[0], pad_idx[1])
        idx_all[c] = arr_i.astype(np.int16)
        dst_all[c] = arr_d.astype(np.float32)
        val_all[c] = arr_v.astype(np.float32)

    idx_wrap = np.empty((C, 128, TOTS // 16), dtype=np.int16)
    for c in range(C):
        idx_wrap[c] = np.tile(idx_all[c].reshape(-1, 16).T, (8, 1))
    # [128, TOTK]: [p, t] = value for slot t*128+p
    dstloc = dst_all.reshape(C, TOTK, 128).transpose(0, 2, 1).copy()
    dvdst = val_all.reshape(C, TOTK, 128).transpose(0, 2, 1).copy()

    dinv_f = dinv.astype(np.float32).reshape(C, S)
    dinv_cols = dinv_f.reshape(C, HN, 128).transpose(0, 2, 1).copy()

    return dict(
        N=N, E=E, S=S, NW=NW, HN=HN, G=G, TOTK=TOTK, TOTS=TOTS,
        K=K, chunks=chunks, tile_w=tile_w, tile_v=tile_v,
        first_tile=first_tile, last_tile=last_tile,
        g_of_old=g_of_old, idx_wrap=idx_wrap, dstloc=dstloc, dvdst=dvdst,
        dinv_cols=dinv_cols,
    )


def _build(tc, outs, ins, plan):
    """Emit the Tile program. ins/outs: dicts of DRAM APs."""
    import contextlib

    import concourse.mybir as mybir

    nc = tc.nc
    S, NW, G, TOTK = plan["S"], plan["NW"], plan["G"], plan["TOTK"]
    N, HN = plan["N"], plan["HN"]
    f32 = mybir.dt.float32
    bf16 = mybir.dt.bfloat16
    i16 = mybir.dt.int16
    AF = mybir.ActivationFunctionType
    OP = mybir.AluOpType
    rg = [list(range(C))]

    hs_dram = [nc.dram_tensor(f"hs{l}", [S, D], bf16) for l in range(2)]
    tables_r = [[nc.dram_tensor(f"table{l}_{r}", [G, D], bf16,
                                addr_space="Shared")
                 for l in range(2)] for r in range(REPEAT)]
    bnin = [nc.dram_tensor(f"bnin{l}", [128, 2], f32) for l in range(2)]
    bnout = [nc.dram_tensor(f"bnout{l}", [128, 2], f32, addr_space="Shared")
             for l in range(2)]

    ctx = contextlib.ExitStack()
    with ctx:
        persist = ctx.enter_context(tc.tile_pool(name="persist", bufs=1))
        gpool = ctx.enter_context(tc.tile_pool(name="gather", bufs=3))
        ppool = ctx.enter_context(tc.tile_pool(name="ptiles", bufs=4))
        spool = ctx.enter_context(tc.tile_pool(name="scratch", bufs=3))
        rowp = ctx.enter_context(tc.tile_pool(name="rows", bufs=3))
        psum_agg = ctx.enter_context(tc.tile_pool(name="psagg", bufs=4, space="PSUM"))
        psum_mm = ctx.enter_context(tc.tile_pool(name="psmm", bufs=2, space="PSUM"))

        aT = persist.tile([128, S], bf16, tag="aT")      # x -> a1
        pre = persist.tile([128, S], bf16, tag="pre")
        osb = persist.tile([128, S], f32, tag="osb")     # final f32 out
        dloc = persist.tile([128, TOTK], f32, tag="dloc")
        dval = persist.tile([128, TOTK], f32, tag="dval")
        idxs = persist.tile([128, plan["TOTS"] // 16], i16, tag="idxs")
        iota = persist.tile([128, WIN], bf16, tag="iota")
        Wt = [persist.tile([128, D], bf16, tag=f"W{l}", name=f"Wt{l}")
              for l in range(2)]
        dcols = persist.tile([128, HN], f32, tag="dcols")
        gbs = persist.tile([128, 4], f32, tag="gbs")
        stats = persist.tile([128, 2 * NW], f32, tag="stats")
        bnsb = [persist.tile([128, 2], f32, tag=f"bnsb{l}", name=f"bnsb{l}")
                for l in range(2)]
        svec = [persist.tile([128, 1], f32, tag=f"svec{l}", name=f"svec{l}")
                for l in range(2)]
        tvec = [persist.tile([128, 1], f32, tag=f"tvec{l}", name=f"tvec{l}")
                for l in range(2)]

        nc.sync.dma_start(out=aT[:], in_=ins["xT"][:, :])
        nc.sync.dma_start(out=dloc[:], in_=ins["dstloc"][:, :])
        nc.sync.dma_start(out=dval[:], in_=ins["dvdst"][:, :])
        nc.sync.dma_start(out=idxs[:], in_=ins["idxw"][:, :])
        nc.sync.dma_start(out=iota[:], in_=ins["iota"][:, :])
        nc.sync.dma_start(out=dcols[:], in_=ins["dinv_cols"][:, :])
        nc.sync.dma_start(out=gbs[:], in_=ins["gb"][:, :])
        nc.sync.dma_start(out=Wt[0][:], in_=ins["W1"][:, :])
        nc.sync.dma_start(out=Wt[1][:], in_=ins["W2"][:, :])

        def phase_mm(l, src_tile):
            """hs{l} = (a @ W) * dinv[src] rows -> DRAM, bf16."""
            for h in range(HN):
                sl = slice(h * 128, (h + 1) * 128)
                psA = psum_mm.tile([128, D], f32, tag="psA")
                nc.tensor.matmul(out=psA[:], lhsT=src_tile[:, sl], rhs=Wt[l][:],
                                 start=True, stop=True)
                hrow = rowp.tile([128, D], bf16, tag="hrow")
                nc.vector.tensor_scalar(out=hrow[:], in0=psA[:],
                                        scalar1=dcols[:, h:h + 1], scalar2=None,
                                        op0=OP.mult)
                nc.sync.dma_start(out=hs_dram[l][sl, :], in_=hrow[:])

        def win_drain(w, psw_w):
            wsl = slice(w * WIN, (w + 1) * WIN)
            nc.scalar.activation(out=pre[:, wsl], in_=psw_w[:], func=AF.Copy,
                                 accum_out=stats[:, w:w + 1])
            sq = spool.tile([128, WIN], bf16, tag="sq")
            nc.scalar.activation(out=sq[:], in_=pre[:, wsl], func=AF.Square,
                                 accum_out=stats[:, NW + w:NW + w + 1])

        def phase_agg(l):
            lo_ap = table[l][0:IDX_LIMIT, :]
            hi_ap = table[l][G - IDX_LIMIT:G, :]
            psw = {}
            for ci, (t0, nt, v) in enumerate(plan["chunks"]):
                n_idx = nt * 128
                gb = gpool.tile([128, CH * 128], bf16, tag="gbuf")
                if not ABLATE_GATHER:
                    out3d = gb[:, :n_idx].rearrange("p (k f) -> p k f", f=D)
                    nc.gpsimd.dma_gather(
                        out_ap=out3d,
                        in_ap=(lo_ap if v == 0 else hi_ap),
                        idxs_ap=idxs[:, t0 * 8:(t0 + nt) * 8],
                        num_idxs=n_idx, num_idxs_reg=n_idx, elem_size=D,
                    )
                else:
                    nc.vector.memset(gb[:, 0:1], 0.0)
                for k in range(nt):
                    t = t0 + k
                    w = int(plan["tile_w"][t])
                    if plan["first_tile"][w] == t:
                        psw[w] = psum_agg.tile([128, WIN], f32, tag="psw",
                                               name=f"psw{w}")
                    P = ppool.tile([128, WIN], bf16, tag="P")
                    if not ABLATE_P:
                        nc.vector.tensor_scalar(
                            out=P[:], in0=iota[:], scalar1=dloc[:, t:t + 1],
                            scalar2=dval[:, t:t + 1], op0=OP.is_equal, op1=OP.mult)
                    else:
                        nc.vector.memset(P[:, 0:1], 0.0)
                    if not ABLATE_MM:
                        nc.tensor.matmul(
                            out=psw[w][:],
                            lhsT=gb[:, k * 128:(k + 1) * 128],
                            rhs=P[:],
                            start=(plan["first_tile"][w] == t),
                            stop=(plan["last_tile"][w] == t))
                    elif plan["first_tile"][w] == t:
                        nc.vector.memset(psw[w][:], 0.0)
                    if plan["last_tile"][w] == t:
                        win_drain(w, psw.pop(w))

        def phase_bn(l, dst_tile):
            ssum = spool.tile([128, 1], f32, tag="bns")
            ssq = spool.tile([128, 1], f32, tag="bnq")
            nc.vector.tensor_reduce(out=ssum[:], in_=stats[:, 0:NW],
                                    axis=mybir.AxisListType.X, op=OP.add)
            nc.vector.tensor_reduce(out=ssq[:], in_=stats[:, NW:2 * NW],
                                    axis=mybir.AxisListType.X, op=OP.add)
            pk = spool.tile([128, 2], f32, tag="bnpack")
            nc.vector.tensor_copy(out=pk[:, 0:1], in_=ssum[:])
            nc.vector.tensor_copy(out=pk[:, 1:2], in_=ssq[:])
            nc.sync.dma_start(out=bnin[l][:, :], in_=pk[:])
            if not ABLATE_AR:
                nc.gpsimd.collective_compute(
                    "AllReduce", OP.add, replica_groups=rg,
                    ins=[bnin[l].ap()], outs=[bnout[l].ap()])
                nc.sync.dma_start(out=bnsb[l][:], in_=bnout[l][:, :])
            else:
                nc.sync.dma_start(out=bnsb[l][:], in_=bnin[l][:, :])
            st = bnsb[l]
            mean = spool.tile([128, 1], f32, tag="mean")
            var = spool.tile([128, 1], f32, tag="var")
            nc.vector.tensor_scalar(out=mean[:], in0=st[:, 0:1],
                                    scalar1=1.0 / N, scalar2=None, op0=OP.mult)
            nc.vector.tensor_scalar(out=var[:], in0=st[:, 1:2],
                                    scalar1=1.0 / N, scalar2=None, op0=OP.mult)
            m2 = spool.tile([128, 1], f32, tag="m2")
            nc.vector.tensor_tensor(out=m2[:], in0=mean[:], in1=mean[:],
                                    op=OP.mult)
            nc.vector.tensor_sub(out=var[:], in0=var[:], in1=m2[:])
            nc.vector.tensor_scalar(out=var[:], in0=var[:], scalar1=1e-5,
                                    scalar2=None, op0=OP.add)
            sd = spool.tile([128, 1], f32, tag="sd")
            nc.scalar.sqrt(out=sd[:], in_=var[:])
            rsd = spool.tile([128, 1], f32, tag="rsd")
            nc.vector.reciprocal(out=rsd[:], in_=sd[:])
            nc.vector.tensor_tensor(out=svec[l][:], in0=gbs[:, 2 * l:2 * l + 1],
                                    in1=rsd[:], op=OP.mult)
            ms = spool.tile([128, 1], f32, tag="ms")
            nc.vector.tensor_tensor(out=ms[:], in0=mean[:], in1=svec[l][:],
                                    op=OP.mult)
            nc.vector.tensor_sub(out=tvec[l][:], in0=gbs[:, 2 * l + 1:2 * l + 2],
                                 in1=ms[:])
            nc.scalar.activation(out=dst_tile[:, :], in_=pre[:, :],
                                 func=AF.Relu, bias=tvec[l][:], scale=svec[l][:])

        for _r in range(REPEAT):
            table = tables_r[_r]
            # layer 1
            phase_mm(0, aT)
            if not ABLATE_AG:
                nc.gpsimd.collective_compute(
                    "AllGather", OP.bypass, replica_groups=rg,
                    ins=[hs_dram[0].ap()], outs=[table[0].ap()])
            phase_agg(0)
            phase_bn(0, aT)                # aT := a1 (bf16)
            # layer 2
            phase_mm(1, aT)
            if not ABLATE_AG:
                nc.gpsimd.collective_compute(
                    "AllGather", OP.bypass, replica_groups=rg,
                    ins=[hs_dram[1].ap()], outs=[table[1].ap()])
            phase_agg(1)
            phase_bn(1, osb)               # osb := relu(bn(pre2)) f32
        nc.sync.dma_start(out=outs["outT"][:, :], in_=osb[:])


def _np_bf16():
    import ml_dtypes
    return np.dtype(ml_dtypes.bfloat16)


def _make_inputs(plan, x, W1, W2, gamma1, beta1, gamma2, beta2):
    S, G = plan["S"], plan["G"]
    bf = _np_bf16()
    xg = np.zeros((G, D), dtype=np.float32)
    xg[plan["g_of_old"]] = np.asarray(x, dtype=np.float32)
    iota = np.tile(np.arange(WIN, dtype=np.float32), (128, 1)).astype(bf)
    gb = np.stack([np.asarray(gamma1, np.float32), np.asarray(beta1, np.float32),
                   np.asarray(gamma2, np.float32), np.asarray(beta2, np.float32)],
                  axis=1).copy()
    in_maps = []
    for c in range(C):
        in_maps.append({
            "xT": np.ascontiguousarray(xg[c * S:(c + 1) * S].T).astype(bf),
            "idxw": plan["idx_wrap"][c],
            "dstloc": plan["dstloc"][c],
            "dvdst": plan["dvdst"][c],
            "dinv_cols": plan["dinv_cols"][c],
            "W1": np.ascontiguousarray(np.asarray(W1, np.float32)).astype(bf),
            "W2": np.ascontiguousarray(np.asarray(W2, np.float32)).astype(bf),
            "gb": gb, "iota": iota,
        })
    return in_maps


def _declare_io(nc, plan):
    import concourse.mybir as mybir
    f32, i16 = mybir.dt.float32, mybir.dt.int16
    bf16 = mybir.dt.bfloat16
    S, TOTK, TOTS, HN = plan["S"], plan["TOTK"], plan["TOTS"], plan["HN"]
    ins = {
        "xT": nc.dram_tensor("xT", [128, S], bf16, kind="ExternalInput"),
        "idxw": nc.dram_tensor("idxw", [128, TOTS // 16], i16,
                               kind="ExternalInput"),
        "dstloc": nc.dram_tensor("dstloc", [128, TOTK], f32,
                                 kind="ExternalInput"),
        "dvdst": nc.dram_tensor("dvdst", [128, TOTK], f32,
                                kind="ExternalInput"),
        "dinv_cols": nc.dram_tensor("dinv_cols", [128, HN], f32,
                                    kind="ExternalInput"),
        "W1": nc.dram_tensor("W1", [128, D], bf16, kind="ExternalInput"),
        "W2": nc.dram_tensor("W2", [128, D], bf16, kind="ExternalInput"),
        "gb": nc.dram_tensor("gb", [128, 4], f32, kind="ExternalInput"),
        "iota": nc.dram_tensor("iota", [128, WIN], bf16, kind="ExternalInput"),
    }
    outs = {"outT": nc.dram_tensor("outT", [128, S], f32,
                                   kind="ExternalOutput")}
    return ins, outs


def _compile(plan, trace_sim=False):
    from concourse import bacc, tile

    nc = bacc.Bacc("TRN2", target_bir_lowering=False, debug=False,
                   num_devices=C)
    ins, outs = _declare_io(nc, plan)
    with tile.TileContext(nc, trace_sim=trace_sim) as tc:
        _build(tc, outs, ins, plan)
    nc.compile()
    return nc


def _run(x, edge_index, W1, b1, gamma1, beta1, W2, b2, gamma2, beta2,
         trace=False):
    from concourse.bass_utils import run_bass_kernel_spmd

    N = x.shape[0]
    plan = _plan(np.asarray(edge_index), N)
    key = (N, plan["E"], plan["TOTK"], tuple(int(k) for k in plan["K"].ravel()))
    if key not in _cache:
        _cache.clear()
        _cache[key] = _compile(plan)
    nc = _cache[key]
    in_maps = _make_inputs(plan, x, W1, W2, gamma1, beta1, gamma2, beta2)
    res = run_bass_kernel_spmd(nc, in_maps, core_ids=list(range(C)), trace=trace)
    S, G = plan["S"], plan["G"]
    aT_full = np.concatenate([res.results[c]["outT"] for c in range(C)], axis=1)
    assert aT_full.shape == (128, G)
    y = np.ascontiguousarray(aT_full.T[plan["g_of_old"]], dtype=np.float32)
    return y, res


def kernel(**inputs):
    y, _ = _run(**inputs)
    return y


# revision 11
# speedup vs baseline: 32.5308x; 1.1234x over previous
"""2-layer GCN (PyG GCNConv + BN + ReLU) on 8 Trainium2 NeuronCores — v2.

Strategy (node sharding; all exchange in bf16):
  - Nodes sorted by in-degree (desc), dealt round-robin to 8 cores; each
    core owns S slots (real + dead all-zero rows). Self-loops are added as
    ordinary edges (one per slot), so the aggregation psum IS the full
    pre-activation: no separate self-loop term.
  - Per layer: each core computes hs = (a @ W)*dinv[src] for its shard in
    bf16, AllGather -> full bf16 table [G,128] in DRAM (ncfw, ~250us).
  - Edges partitioned by destination core, grouped into 128-dst windows;
    per 128-edge tile: dma_gather bf16 rows (256B each), build
    P[slot,dst] = (dstloc==iota) * dinv[dst] in ONE bf16 tensor_scalar
    (4x DVE mode), accumulate gathered^T @ P into PSUM [128 feat, 128 dst].
  - Window drain on ACT: Copy psum->pre (bf16) with accum_out=sum, then
    Square with accum_out=sumsq. BN stats via tiny AllReduce (~14us);
    y = relu(pre*s + t) in ONE whole-row ACT op (per-partition scale/bias).
  - Output returned transposed [128, S] f32 per core; host reassembles.

dma_gather indices are int16 (<32768): the G-row table is addressed via
two overlapping views lo=[0,32768) / hi=[G-32768,G); segment padding
points at dead rows (core0/core7 tail slots, always padding => hs rows 0).
b1/b2 dropped (BN right after +b is invariant to per-feature shifts).
"""

import numpy as np

C = 8            # cores
D = 128          # feature dim
WIN = 128        # dst nodes per aggregation window (psum free dim)
CH = 8           # max gather tiles (of 128 slots) per dma_gather call
IDX_LIMIT = 32768
REPEAT = 1       # dev knob: repeat the 2-layer body for slope timing
ABLATE_GATHER = False   # dev: skip dma_gather (wrong results, timing only)
ABLATE_P = False        # dev: skip P tensor_scalar build
ABLATE_MM = False       # dev: skip aggregation matmuls
ABLATE_AG = False       # dev: skip the table AllGather
ABLATE_AR = False       # dev: skip the BN AllReduce (wrong stats, timing only)

_cache = {}


def _plan(edge_index, N):
    """Host-side graph preprocessing -> per-core arrays + static structure."""
    src = np.asarray(edge_index[0], dtype=np.int64)
    dst = np.asarray(edge_index[1], dtype=np.int64)
    E = src.shape[0]

    Nr = -(-N // C)                      # real nodes per core
    S = -(-Nr // WIN) * WIN              # padded slots per core
    NW = S // WIN
    HN = S // 128
    G = C * S                            # table rows
    assert G > IDX_LIMIT and S - Nr >= 1, (G, S, Nr)

    deg = np.bincount(dst, minlength=N).astype(np.int64) + 1
    order = np.argsort(-deg, kind="stable")        # rank -> old id
    ranks = np.arange(N, dtype=np.int64)
    g_of_old = np.empty(N, dtype=np.int64)
    g_of_old[order] = (ranks % C) * S + ranks // C

    dinv = np.zeros(G, dtype=np.float64)
    dinv[g_of_old] = deg.astype(np.float64) ** -0.5

    # self-loops as ordinary edges: one per slot (padding slots too; their
    # rows are dead/zero and dinv=0 so they contribute exactly 0)
    gall = np.arange(G, dtype=np.int64)
    gs = np.concatenate([g_of_old[src], gall])
    gd = np.concatenate([g_of_old[dst], gall])

    core_e = gd // S
    w_e = (gd % S) // WIN
    dl_e = (gd % S) % WIN
    view_e = (gs >= IDX_LIMIT).astype(np.int64)
    idx_e = np.where(view_e == 0, gs, gs - (G - IDX_LIMIT))
    assert idx_e.max() < IDX_LIMIT and idx_e.min() >= 0
    val_e = dinv[gd]                     # dinv[dst] folded into the one-hot

    pad_idx = (S - 1, IDX_LIMIT - 1)     # core0 dead row (lo) / core7 dead (hi)

    counts = np.zeros((C, NW, 2), dtype=np.int64)
    np.add.at(counts, (core_e, w_e, view_e), 1)
    K = -(-counts.max(axis=0) // 128)             # [NW, 2] unified tile counts

    tile_w, tile_v = [], []
    for w in range(NW):
        tile_w += [w] * int(K[w, 0]); tile_v += [0] * int(K[w, 0])
        tile_w += [w] * int(K[w, 1]); tile_v += [1] * int(K[w, 1])
    tile_w = np.array(tile_w, dtype=np.int64)
    tile_v = np.array(tile_v, dtype=np.int64)
    TOTK = len(tile_w)
    TOTS = TOTK * 128

    chunks = []   # (tile0, ntiles, view) — runs of one view, <= CH tiles
    t = 0
    while t < TOTK:
        v = tile_v[t]
        r = t
        while r < TOTK and tile_v[r] == v and r - t < CH:
            r += 1
        chunks.append((int(t), int(r - t), int(v)))
        t = r
    first_tile, last_tile = {}, {}
    for t in range(TOTK):
        w = int(tile_w[t])
        first_tile.setdefault(w, t)
        last_tile[w] = t
    assert len(first_tile) == NW          # every window has >=1 tile

    tile_base = np.zeros((NW, 2), dtype=np.int64)
    b = 0
    for w in range(NW):
        for v in range(2):
            tile_base[w, v] = b
            b += int(K[w, v]) * 128
    slot_view = np.empty(TOTS, dtype=np.int64)
    for w in range(NW):
        for v in range(2):
            s0 = int(tile_base[w, v]); n = int(K[w, v]) * 128
            slot_view[s0:s0 + n] = v

    ord_e = np.lexsort((dl_e, view_e, w_e, core_e))
    src_s = idx_e[ord_e]; core_s = core_e[ord_e]
    w_s = w_e[ord_e]; v_s = view_e[ord_e]; dl_s = dl_e[ord_e]
    val_s = val_e[ord_e]

    idx_all = np.empty((C, TOTS), dtype=np.int16)
    dst_all = np.zeros((C, TOTS), dtype=np.float32)
    val_all = np.zeros((C, TOTS), dtype=np.float32)
    for c in range(C):
        m = core_s == c
        iw, iv, ii, idl, ival = w_s[m], v_s[m], src_s[m], dl_s[m], val_s[m]
        arr_i = np.full(TOTS, -1, dtype=np.int64)
        arr_d = np.zeros(TOTS, dtype=np.int64)
        arr_v = np.zeros(TOTS, dtype=np.float64)
        if len(iw):
            seg_id = iw * 2 + iv
            change = np.r_[True, np.diff(seg_id) != 0]
            seg_start = np.maximum.accumulate(
                np.where(change, np.arange(len(seg_id)), 0))
            within = np.arange(len(seg_id)) - seg_start
            flat = tile_base[iw, iv] + within
            arr_i[flat] = ii
            arr_d[flat] = idl
            arr_v[flat] = ival
        padm = arr_i == -1
        arr_i[padm] = np.where(slot_view[padm] == 0, pad_idx[0], pad_idx[1])
        idx_all[c] = arr_i.astype(np.int16)
        dst_all[c] = arr_d.astype(np.float32)
        val_all[c] = arr_v.astype(np.float32)

    idx_wrap = np.empty((C, 128, TOTS // 16), dtype=np.int16)
    for c in range(C):
        idx_wrap[c] = np.tile(idx_all[c].reshape(-1, 16).T, (8, 1))
    # [128, TOTK]: [p, t] = value for slot t*128+p
    dstloc = dst_all.reshape(C, TOTK, 128).transpose(0, 2, 1).copy()
    dvdst = val_all.reshape(C, TOTK, 128).transpose(0, 2, 1).copy()

    dinv_f = dinv.astype(np.float32).reshape(C, S)
    dinv_cols = dinv_f.reshape(C, HN, 128).transpose(0, 2, 1).copy()

    return dict(
        N=N, E=E, S=S, NW=NW, HN=HN, G=G, TOTK=TOTK, TOTS=TOTS,
        K=K, chunks=chunks, tile_w=tile_w, tile_v=tile_v,
        first_tile=first_tile, last_tile=last_tile,
        g_of_old=g_of_old, idx_wrap=idx_wrap, dstloc=dstloc, dvdst=dvdst,
        dinv_cols=dinv_cols,
    )


def _build(tc, outs, ins, plan):
    """Emit the Tile program. ins/outs: dicts of DRAM APs."""
    import contextlib

    import concourse.mybir as mybir

    nc = tc.nc
    S, NW, G, TOTK = plan["S"], plan["NW"], plan["G"], plan["TOTK"]
    N, HN = plan["N"], plan["HN"]
    f32 = mybir.dt.float32
    bf16 = mybir.dt.bfloat16
    i16 = mybir.dt.int16
    AF = mybir.ActivationFunctionType
    OP = mybir.AluOpType
    rg = [list(range(C))]

    hs_dram = [nc.dram_tensor(f"hs{l}", [S, D], bf16) for l in range(2)]
    tables_r = [[nc.dram_tensor(f"table{l}_{r}", [G, D], bf16,
                                addr_space="Shared")
                 for l in range(2)] for r in range(REPEAT)]
    bnin = [nc.dram_tensor(f"bnin{l}", [128, 2], f32) for l in range(2)]
    bnout = [nc.dram_tensor(f"bnout{l}", [128, 2], f32, addr_space="Shared")
             for l in range(2)]

    ctx = contextlib.ExitStack()
    with ctx:
        persist = ctx.enter_context(tc.tile_pool(name="persist", bufs=1))
        gpool = ctx.enter_context(tc.tile_pool(name="gather", bufs=3))
        ppool = ctx.enter_context(tc.tile_pool(name="ptiles", bufs=4))
        spool = ctx.enter_context(tc.tile_pool(name="scratch", bufs=3))
        rowp = ctx.enter_context(tc.tile_pool(name="rows", bufs=3))
        psum_agg = ctx.enter_context(tc.tile_pool(name="psagg", bufs=4, space="PSUM"))
        psum_mm = ctx.enter_context(tc.tile_pool(name="psmm", bufs=2, space="PSUM"))

        aT = persist.tile([128, S], bf16, tag="aT")      # x -> a1
        pre = persist.tile([128, S], bf16, tag="pre")
        osb = persist.tile([128, S], f32, tag="osb")     # final f32 out
        dloc = persist.tile([128, TOTK], f32, tag="dloc")
        dval = persist.tile([128, TOTK], f32, tag="dval")
        idxs = persist.tile([128, plan["TOTS"] // 16], i16, tag="idxs")
        iota = persist.tile([128, WIN], bf16, tag="iota")
        Wt = [persist.tile([128, D], bf16, tag=f"W{l}", name=f"Wt{l}")
              for l in range(2)]
        dcols = persist.tile([128, HN], f32, tag="dcols")
        gbs = persist.tile([128, 4], f32, tag="gbs")
        stats = persist.tile([128, 2 * NW], f32, tag="stats")
        bnsb = [persist.tile([128, 2], f32, tag=f"bnsb{l}", name=f"bnsb{l}")
                for l in range(2)]
        svec = [persist.tile([128, 1], f32, tag=f"svec{l}", name=f"svec{l}")
                for l in range(2)]
        tvec = [persist.tile([128, 1], f32, tag=f"tvec{l}", name=f"tvec{l}")
                for l in range(2)]

        nc.sync.dma_start(out=aT[:], in_=ins["xT"][:, :])
        nc.sync.dma_start(out=dloc[:], in_=ins["dstloc"][:, :])
        nc.sync.dma_start(out=dval[:], in_=ins["dvdst"][:, :])
        nc.sync.dma_start(out=idxs[:], in_=ins["idxw"][:, :])
        nc.sync.dma_start(out=iota[:], in_=ins["iota"][:, :])
        nc.sync.dma_start(out=dcols[:], in_=ins["dinv_cols"][:, :])
        nc.sync.dma_start(out=gbs[:], in_=ins["gb"][:, :])
        nc.sync.dma_start(out=Wt[0][:], in_=ins["W1"][:, :])
        nc.sync.dma_start(out=Wt[1][:], in_=ins["W2"][:, :])

        def phase_mm(l, src_tile):
            """hs{l} = (a @ W) * dinv[src] rows -> DRAM, bf16."""
            for h in range(HN):
                sl = slice(h * 128, (h + 1) * 128)
                psA = psum_mm.tile([128, D], f32, tag="psA")
                nc.tensor.matmul(out=psA[:], lhsT=src_tile[:, sl], rhs=Wt[l][:],
                                 start=True, stop=True)
                hrow = rowp.tile([128, D], bf16, tag="hrow")
                nc.vector.tensor_scalar(out=hrow[:], in0=psA[:],
                                        scalar1=dcols[:, h:h + 1], scalar2=None,
                                        op0=OP.mult)
                nc.sync.dma_start(out=hs_dram[l][sl, :], in_=hrow[:])

        def win_drain(w, psw_w):
            wsl = slice(w * WIN, (w + 1) * WIN)
            nc.scalar.activation(out=pre[:, wsl], in_=psw_w[:], func=AF.Copy,
                                 accum_out=stats[:, w:w + 1])
            sq = spool.tile([128, WIN], bf16, tag="sq")
            nc.scalar.activation(out=sq[:], in_=pre[:, wsl], func=AF.Square,
                                 accum_out=stats[:, NW + w:NW + w + 1])

        def phase_agg(l):
            lo_ap = table[l][0:IDX_LIMIT, :]
            hi_ap = table[l][G - IDX_LIMIT:G, :]
            psw = {}
            for ci, (t0, nt, v) in enumerate(plan["chunks"]):
                n_idx = nt * 128
                gb = gpool.tile([128, CH * 128], bf16, tag="gbuf")
                if not ABLATE_GATHER:
                    out3d = gb[:, :n_idx].rearrange("p (k f) -> p k f", f=D)
                    nc.gpsimd.dma_gather(
                        out_ap=out3d,
                        in_ap=(lo_ap if v == 0 else hi_ap),
                        idxs_ap=idxs[:, t0 * 8:(t0 + nt) * 8],
                        num_idxs=n_idx, num_idxs_reg=n_idx, elem_size=D,
                    )
                else:
                    nc.vector.memset(gb[:, 0:1], 0.0)
                for k in range(nt):
                    t = t0 + k
                    w = int(plan["tile_w"][t])
                    if plan["first_tile"][w] == t:
                        psw[w] = psum_agg.tile([128, WIN], f32, tag="psw",
                                               name=f"psw{w}")
                    P = ppool.tile([128, WIN], bf16, tag="P")
                    if not ABLATE_P:
                        nc.vector.tensor_scalar(
                            out=P[:], in0=iota[:], scalar1=dloc[:, t:t + 1],
                            scalar2=dval[:, t:t + 1], op0=OP.is_equal, op1=OP.mult)
                    else:
                        nc.vector.memset(P[:, 0:1], 0.0)
                    if not ABLATE_MM:
                        nc.tensor.matmul(
                            out=psw[w][:],
                            lhsT=gb[:, k * 128:(k + 1) * 128],
                            rhs=P[:],
                            start=(plan["first_tile"][w] == t),
                            stop=(plan["last_tile"][w] == t))
                    elif plan["first_tile"][w] == t:
                        nc.vector.memset(psw[w][:], 0.0)
                    if plan["last_tile"][w] == t:
                        win_drain(w, psw.pop(w))

        def phase_bn(l, dst_tile):
            ssum = spool.tile([128, 1], f32, tag="bns")
            ssq = spool.tile([128, 1], f32, tag="bnq")
            nc.vector.tensor_reduce(out=ssum[:], in_=stats[:, 0:NW],
                                    axis=mybir.AxisListType.X, op=OP.add)
            nc.vector.tensor_reduce(out=ssq[:], in_=stats[:, NW:2 * NW],
                                    axis=mybir.AxisListType.X, op=OP.add)
            pk = spool.tile([128, 2], f32, tag="bnpack")
            nc.vector.tensor_copy(out=pk[:, 0:1], in_=ssum[:])
            nc.vector.tensor_copy(out=pk[:, 1:2], in_=ssq[:])
            nc.sync.dma_start(out=bnin[l][:, :], in_=pk[:])
            if not ABLATE_AR:
                nc.gpsimd.collective_compute(
                    "AllReduce", OP.add, replica_groups=rg,
                    ins=[bnin[l].ap()], outs=[bnout[l].ap()])
                nc.sync.dma_start(out=bnsb[l][:], in_=bnout[l][:, :])
            else:
                nc.sync.dma_start(out=bnsb[l][:], in_=bnin[l][:, :])
            st = bnsb[l]
            mean = spool.tile([128, 1], f32, tag="mean")
            var = spool.tile([128, 1], f32, tag="var")
            nc.vector.tensor_scalar(out=mean[:], in0=st[:, 0:1],
                                    scalar1=1.0 / N, scalar2=None, op0=OP.mult)
            nc.vector.tensor_scalar(out=var[:], in0=st[:, 1:2],
                                    scalar1=1.0 / N, scalar2=None, op0=OP.mult)
            m2 = spool.tile([128, 1], f32, tag="m2")
            nc.vector.tensor_tensor(out=m2[:], in0=mean[:], in1=mean[:],
                                    op=OP.mult)
            nc.vector.tensor_sub(out=var[:], in0=var[:], in1=m2[:])
            nc.vector.tensor_scalar(out=var[:], in0=var[:], scalar1=1e-5,
                                    scalar2=None, op0=OP.add)
            sd = spool.tile([128, 1], f32, tag="sd")
            nc.scalar.sqrt(out=sd[:], in_=var[:])
            rsd = spool.tile([128, 1], f32, tag="rsd")
            nc.vector.reciprocal(out=rsd[:], in_=sd[:])
            nc.vector.tensor_tensor(out=svec[l][:], in0=gbs[:, 2 * l:2 * l + 1],
                                    in1=rsd[:], op=OP.mult)
            ms = spool.tile([128, 1], f32, tag="ms")
            nc.vector.tensor_tensor(out=ms[:], in0=mean[:], in1=svec[l][:],
                                    op=OP.mult)
            nc.vector.tensor_sub(out=tvec[l][:], in0=gbs[:, 2 * l + 1:2 * l + 2],
                                 in1=ms[:])
            nc.scalar.activation(out=dst_tile[:, :], in_=pre[:, :],
                                 func=AF.Relu, bias=tvec[l][:], scale=svec[l][:])

        for _r in range(REPEAT):
            table = tables_r[_r]
            # layer 1
            phase_mm(0, aT)
            if not ABLATE_AG:
                nc.gpsimd.collective_compute(
                    "AllGather", OP.bypass, replica_groups=rg,
                    ins=[hs_dram[0].ap()], outs=[table[0].ap()])
            phase_agg(0)
            phase_bn(0, aT)                # aT := a1 (bf16)
            # layer 2
            phase_mm(1, aT)
            if not ABLATE_AG:
                nc.gpsimd.collective_compute(
                    "AllGather", OP.bypass, replica_groups=rg,
                    ins=[hs_dram[1].ap()], outs=[table[1].ap()])
            phase_agg(1)
            phase_bn(1, osb)               # osb := relu(bn(pre2)) f32
        nc.sync.dma_start(out=outs["outT"][:, :], in_=osb[:])


def _np_bf16():
    import ml_dtypes
    return np.dtype(ml_dtypes.bfloat16)


def _make_inputs(plan, x, W1, W2, gamma1, beta1, gamma2, beta2):
    S, G = plan["S"], plan["G"]
    bf = _np_bf16()
    xg = np.zeros((G, D), dtype=np.float32)
    xg[plan["g_of_old"]] = np.asarray(x, dtype=np.float32)
    iota = np.tile(np.arange(WIN, dtype=np.float32), (128, 1)).astype(bf)
    gb = np.stack([np.asarray(gamma1, np.float32), np.asarray(beta1, np.float32),
                   np.asarray(gamma2, np.float32), np.asarray(beta2, np.float32)],
                  axis=1).copy()
    in_maps = []
    for c in range(C):
        in_maps.append({
            "xT": np.ascontiguousarray(xg[c * S:(c + 1) * S].T).astype(bf),
            "idxw": plan["idx_wrap"][c],
            "dstloc": plan["dstloc"][c],
            "dvdst": plan["dvdst"][c],
            "dinv_cols": plan["dinv_cols"][c],
            "W1": np.ascontiguousarray(np.asarray(W1, np.float32)).astype(bf),
            "W2": np.ascontiguousarray(np.asarray(W2, np.float32)).astype(bf),
            "gb": gb, "iota": iota,
        })
    return in_maps


def _declare_io(nc, plan):
    import concourse.mybir as mybir
    f32, i16 = mybir.dt.float32, mybir.dt.int16
    bf16 = mybir.dt.bfloat16
    S, TOTK, TOTS, HN = plan["S"], plan["TOTK"], plan["TOTS"], plan["HN"]
    ins = {
        "xT": nc.dram_tensor("xT", [128, S], bf16, kind="ExternalInput"),
        "idxw": nc.dram_tensor("idxw", [128, TOTS // 16], i16,
                               kind="ExternalInput"),
        "dstloc": nc.dram_tensor("dstloc", [128, TOTK], f32,
                                 kind="ExternalInput"),
        "dvdst": nc.dram_tensor("dvdst", [128, TOTK], f32,
                                kind="ExternalInput"),
        "dinv_cols": nc.dram_tensor("dinv_cols", [128, HN], f32,
                                    kind="ExternalInput"),
        "W1": nc.dram_tensor("W1", [128, D], bf16, kind="ExternalInput"),
        "W2": nc.dram_tensor("W2", [128, D], bf16, kind="ExternalInput"),
        "gb": nc.dram_tensor("gb", [128, 4], f32, kind="ExternalInput"),
        "iota": nc.dram_tensor("iota", [128, WIN], bf16, kind="ExternalInput"),
    }
    outs = {"outT": nc.dram_tensor("outT", [128, S], f32,
                                   kind="ExternalOutput")}
    return ins, outs


def _compile(plan, trace_sim=False):
    from concourse import bacc, tile

    nc = bacc.Bacc("TRN2", target_bir_lowering=False, debug=False,
                   num_devices=C)
    ins, outs = _declare_io(nc, plan)
    with tile.TileContext(nc, trace_sim=trace_sim) as tc:
        _build(tc, outs, ins, plan)
    nc.compile()
    return nc


def _run(x, edge_index, W1, b1, gamma1, beta1, W2, b2, gamma2, beta2,
         trace=False):
    from concourse.bass_utils import run_bass_kernel_spmd

    N = x.shape[0]
    plan = _plan(np.asarray(edge_index), N)
    key = (N, plan["E"], plan["TOTK"], tuple(int(k) for k in plan["K"].ravel()))
    if key not in _cache:
        _cache.clear()
        _cache[key] = _compile(plan)
    nc = _cache[key]
    in_maps = _make_inputs(plan, x, W1, W2, gamma1, beta1, gamma2, beta2)
    res = run_bass_kernel_spmd(nc, in_maps, core_ids=list(range(C)), trace=trace)
    S, G = plan["S"], plan["G"]
    aT_full = np.concatenate([res.results[c]["outT"] for c in range(C)], axis=1)
    assert aT_full.shape == (128, G)
    y = np.ascontiguousarray(aT_full.T[plan["g_of_old"]], dtype=np.float32)
    return y, res


def kernel(**inputs):
    y, _ = _run(**inputs)
    return y
